# revision 1
# baseline (speedup 1.0000x reference)
"""CharEmb kernel for Trainium2 (8 NeuronCores, batch-sharded).

Computation (per word of 32 chars):
  emb = table[ids]                  # [32 chars, 64] per word
  x[i, j] = emb[i//2, 32*(i%2)+j]   # raw-buffer reshape [64, 32]
  y[f, t] = sum_{i,k} x[i, t+k] * w[f, i, k]   (valid conv, K=3)
  out[f] = max_t y[f, t] + b[f]

The baseline's GPSIMD dma_gather (8ns/descriptor, 526us serial) is
replaced by a tensor-engine one-hot gather. The one-hot over the 101-row
table is precomputed on host (like the baseline's index reformatting)
and DMA-streamed. Per chunk of 32 words (1024 chars):
  1. DMA: one-hot [101, 1024] bf16 from DRAM (host-permuted column
     order: within each 512-col half, col 64*(w//2) + 2p + (w%2) holds
     char (w, p) so the packed transpose lands in conv-friendly form).
  2. PE: two gather matmuls, table bf16 [101, 64] stationary ->
     PSUM [128, 512] fp32 (the two 512-col halves stacked on partitions).
  3. Act: PSUM -> SBUF bf16 copy [128, 512].
  4. DVE: two 32x32 stream transposes on int32-packed views (pairs of
     bf16) -> T_h[32h+p, 64*(w//2) + 2j + w%2] = emb_w[p, 32h+j],
     one [128, 512] T-tile per 16-word half.
  5. DMA: shift-dup rows 64-127 = rows 0-63 shifted +2 bf16 cols
     (tap k=1), one DMA per T-tile per chunk-pair.
  6. PE conv per half: fused taps k=0,1 (128-row contraction) + tap
     k=2 (64 rows); t-window = strided column views. Weight loads are
     k-outer across the chunk pair.
  7. maxpool over t: Act copies y -> SBUF bf16, DVE runs a 2x-mode
     pairwise max tree (every 16th chunk: direct DVE reduce from PSUM).
Finally bias add + store out[f, word] (word-linear).
"""

import sys
from contextlib import ExitStack

import numpy as np

if "/opt/trn_rl_repo" not in sys.path:
    sys.path.insert(0, "/opt/trn_rl_repo")

import concourse.bass as bass
import concourse.tile as tile
from concourse import bacc, mybir
from concourse.bass_utils import run_bass_kernel_spmd

# Problem constants (hardcoded per spec)
B, S, C = 32, 512, 32
V, E = 101, 64
F, K = 128, 3
T = C - K + 1  # 30 valid conv positions
NCORES = 8
WORDS = (B * S) // NCORES  # 2048 words per core
NCHARS = WORDS * C  # 65536

CHUNK_W = 32                 # words per chunk
CH_COLS = CHUNK_W * C        # 1024 chars per chunk
NCHUNKS = WORDS // CHUNK_W   # 64
HALF_W = 16                  # words per T-tile half
HALF_COLS = HALF_W * C       # 512

f32 = mybir.dt.float32
bf16 = mybir.dt.bfloat16
i32 = mybir.dt.int32
i16 = mybir.dt.int16


def build_kernel(num_devices=NCORES):
    nc = bacc.Bacc(
        "TRN2",
        target_bir_lowering=False,
        debug=False,
        enable_asserts=True,
        num_devices=num_devices,
    )

    idx_d = nc.dram_tensor("idx", [1, NCHARS], i16, kind="ExternalInput")
    tab_d = nc.dram_tensor("tab", [V, E], bf16, kind="ExternalInput")
    w_d = nc.dram_tensor("wmat", [128, 256], bf16, kind="ExternalInput")
    b_d = nc.dram_tensor("bias", [128, 1], f32, kind="ExternalInput")
    out_d = nc.dram_tensor("out", [128, WORDS], f32, kind="ExternalOutput")

    with tile.TileContext(nc) as tc, ExitStack() as ctx:
        const_pool = ctx.enter_context(tc.tile_pool(name="const", bufs=1))
        oh_pool = ctx.enter_context(tc.tile_pool(name="oh", bufs=3))
        ids_pool = ctx.enter_context(tc.tile_pool(name="ids", bufs=3))
        gsb_pool = ctx.enter_context(tc.tile_pool(name="gsb", bufs=3))
        ta_pool = ctx.enter_context(tc.tile_pool(name="tta", bufs=2))
        tb_pool = ctx.enter_context(tc.tile_pool(name="ttb", bufs=2))
        ysb_pool = ctx.enter_context(tc.tile_pool(name="ysb", bufs=3))
        g_psum = ctx.enter_context(tc.tile_pool(name="gps", bufs=3, space="PSUM"))
        y_psum = ctx.enter_context(tc.tile_pool(name="yps", bufs=2, space="PSUM"))

        tab_sb = const_pool.tile([V, E], bf16)
        iota_sb = const_pool.tile([128, 1], f32)
        w_sb = const_pool.tile([128, 256], bf16)
        b_sb = const_pool.tile([128, 1], f32)
        obuf = const_pool.tile([128, WORDS], f32)

        nc.sync.dma_start(tab_sb[:], tab_d.ap())
        nc.sync.dma_start(w_sb[:], w_d.ap())
        nc.sync.dma_start(b_sb[:], b_d.ap())
        nc.gpsimd.iota(
            iota_sb[:], pattern=[[1, 1]], base=0, channel_multiplier=1,
            allow_small_or_imprecise_dtypes=True,
        )

        for pp in range(NCHUNKS // 2):  # chunk pairs
            # ids for the pair -> Pool broadcast -> DVE one-hot
            ids_t = ids_pool.tile([128, 2 * CH_COLS], i16)
            nc.sync.dma_start(
                ids_t[0:1, :],
                idx_d.ap()[:, pp * 2 * CH_COLS:(pp + 1) * 2 * CH_COLS],
            )
            nc.gpsimd.partition_broadcast(ids_t[:, :], ids_t[0:1, :])
            oh_t = oh_pool.tile([128, 2 * CH_COLS], bf16)
            nc.vector.tensor_scalar(
                oh_t[0:V, :],
                ids_t[0:V, :],
                iota_sb[0:V, 0:1],
                None,
                op0=mybir.AluOpType.is_equal,
            )

            # T tiles for the pair: cols 512*cp per chunk cp in {0,1}
            t_a = ta_pool.tile([128, 2 * HALF_COLS], bf16)
            t_b = tb_pool.tile([128, 2 * HALF_COLS], bf16)

            g_list = []
            for cp in range(2):
                # gather matmuls -> PSUM [128, 512] (halves stacked)
                g_ps = g_psum.tile([128, HALF_COLS], f32)
                base = pp * 2 * CH_COLS + cp * CH_COLS
                for hh in range(2):
                    nc.tensor.matmul(
                        g_ps[64 * hh:64 * (hh + 1), :],
                        tab_sb[0:V, 0:E],
                        oh_t[0:V, cp * CH_COLS + 512 * hh:
                             cp * CH_COLS + 512 * (hh + 1)],
                        start=True,
                        stop=True,
                    )
                g_list.append(g_ps)

            for cp in range(2):
                g_ps = g_list[cp]
                # PSUM -> SBUF bf16
                gsb = gsb_pool.tile([128, HALF_COLS], bf16)
                nc.scalar.copy(gsb[:, :], g_ps[:, :])
                # packed transposes into the pair tiles
                nc.vector.transpose(
                    t_a[0:64, 512 * cp:512 * (cp + 1)].bitcast(i32),
                    gsb[0:64, :].bitcast(i32),
                )
                nc.vector.transpose(
                    t_b[0:64, 512 * cp:512 * (cp + 1)].bitcast(i32),
                    gsb[64:128, :].bitcast(i32),
                )

            # shift-dup rows 64-127 (+2 bf16 cols), one DMA per tile
            nc.sync.dma_start(t_a[64:128, 0:2 * HALF_COLS - 2],
                              t_a[0:64, 2:2 * HALF_COLS])
            nc.sync.dma_start(t_b[64:128, 0:2 * HALF_COLS - 2],
                              t_b[0:64, 2:2 * HALF_COLS])

            # conv for the pair; W loads k-outer: W01 x4 then W2 x4
            y_list = []
            views = []
            for cp in range(2):
                y_ps = y_psum.tile([128, 2 * 512], f32)
                y_list.append(y_ps)
                for r, t_t in enumerate((t_a, t_b)):
                    tf = (
                        t_t[:, 512 * cp:512 * (cp + 1)]
                        .rearrange("q (u j e) -> q u e j", j=C, e=2)
                    )
                    tl = (
                        t_t[0:64, 512 * cp:512 * (cp + 1)]
                        .rearrange("q (u j e) -> q u e j", j=C, e=2)
                    )
                    out_ap = (
                        y_ps[:, 512 * r:512 * r + HALF_W * T]
                        .rearrange("f (u e t) -> f u e t", t=T, e=2)
                    )
                    views.append((out_ap, tf, tl))
            for (out_ap, tf, tl) in views:
                nc.tensor.matmul(
                    out_ap, w_sb[:, 0:128], tf[:, :, :, 0:T],
                    start=True, stop=False, skip_group_check=True,
                )
            for (out_ap, tf, tl) in views:
                nc.tensor.matmul(
                    out_ap, w_sb[0:64, 128:256], tl[:, :, :, 2:2 + T],
                    start=False, stop=True, skip_group_check=True,
                )

            # maxpool over t -> obuf
            for cp in range(2):
                cc = 2 * pp + cp
                y_ps = y_list[cp]
                red_out = (
                    obuf[:, cc * CHUNK_W:(cc + 1) * CHUNK_W]
                    .rearrange("f (r w) -> f r w", w=HALF_W)
                )
                if cc % 16 == 0:
                    # direct DVE reduce from PSUM
                    red_in = (
                        y_ps[:, :].rearrange("f (r x) -> f r x", x=512)
                        [:, :, 0:HALF_W * T]
                        .rearrange("f r (w t) -> f r w t", t=T)
                    )
                    nc.vector.tensor_reduce(
                        red_out, red_in, axis=mybir.AxisListType.X,
                        op=mybir.AluOpType.max,
                    )
                else:
                    # Act copies y -> SBUF bf16; DVE 2x max tree
                    ysb = ysb_pool.tile([128, 2 * 512], bf16)
                    nc.scalar.copy(ysb[:, :], y_ps[:, :])
                    yv = (
                        ysb[:, :].rearrange("f (r x) -> f r x", x=512)
                        [:, :, 0:HALF_W * T]
                        .rearrange("f r (w t) -> f r w t", t=T)
                    )
                    for lo, hi, n in ((0, 15, 15), (0, 8, 7), (0, 4, 4),
                                      (0, 2, 2)):
                        nc.vector.tensor_tensor(
                            yv[:, :, :, lo:lo + n],
                            yv[:, :, :, lo:lo + n],
                            yv[:, :, :, hi:hi + n],
                            op=mybir.AluOpType.max,
                        )
                    nc.vector.tensor_tensor(
                        red_out.rearrange("f r (w o) -> f r w o", o=1),
                        yv[:, :, :, 0:1],
                        yv[:, :, :, 1:2],
                        op=mybir.AluOpType.max,
                    )

        # bias + store
        nc.vector.tensor_scalar_add(obuf[:], obuf[:], b_sb[:, 0:1])
        nc.sync.dma_start(out_d.ap(), obuf[:])

    nc.compile()
    return nc


def host_prep(char_ids, emb_table, conv_w, conv_b, num_devices=NCORES):
    """Build per-core input maps from full inputs."""
    char_ids = np.asarray(char_ids)
    emb_table = np.asarray(emb_table, dtype=np.float32)
    conv_w = np.asarray(conv_w, dtype=np.float32)
    conv_b = np.asarray(conv_b, dtype=np.float32)

    bf = mybir.dt.np(bf16)
    tab = emb_table.astype(bf)

    # permuted ids: within each 512-col half (16 words), column
    # 64*(w'//2) + 2p + (w'%2) holds char (w', p)
    ids_all = char_ids.reshape(-1, HALF_W, C).astype(np.int64)  # [nh, w', p]
    nh = ids_all.shape[0]
    ids_perm = (
        ids_all.reshape(nh, HALF_W // 2, 2, C)
        .transpose(0, 1, 3, 2)  # [half, u, p, eps]
        .reshape(nh * HALF_COLS)
    )

    ids_perm = ids_perm.astype(np.int16)

    # W layout: q = 32h+p (q<64) -> channel 2p+h
    q = np.arange(64)
    ch = 2 * (q % 32) + q // 32
    wmat = np.zeros((128, 256), dtype=np.float32)
    wmat[0:64, 0:128] = conv_w[:, ch, 0].T
    wmat[64:128, 0:128] = conv_w[:, ch, 1].T
    wmat[0:64, 128:256] = conv_w[:, ch, 2].T
    wmat = wmat.astype(bf)

    bias = conv_b.reshape(128, 1).astype(np.float32)

    in_maps = []
    for jcore in range(num_devices):
        ids_core = ids_perm[jcore * NCHARS:(jcore + 1) * NCHARS]
        in_maps.append(
            {
                "idx": np.ascontiguousarray(ids_core.reshape(1, NCHARS)),
                "tab": tab,
                "wmat": wmat,
                "bias": bias,
            }
        )
    return in_maps


def _ensure_ntff_hook():
    """The agent image's antenv lacks axon_hooks; shim it and install the
    ctypes NTFF profiling hook so trace=True yields HW exec times."""
    import types

    if "antenv.axon_hooks" in sys.modules:
        return
    mod = types.ModuleType("antenv.axon_hooks")
    _hook = [None]
    mod.get_axon_ntff_profile_hook = lambda: _hook[0]
    mod.set_axon_ntff_profile_hook = lambda h: _hook.__setitem__(0, h)
    sys.modules["antenv.axon_hooks"] = mod
    try:
        import antenv

        antenv.axon_hooks = mod
        from trn_agent_boot.trn_boot import _ntff_profile_via_ctypes

        hook = _ntff_profile_via_ctypes("/opt/axon/libaxon_pjrt.so")
        mod.set_axon_ntff_profile_hook(hook)
    except Exception as e:  # degrade to no-trace
        print(f"ntff hook install failed: {e}", file=sys.stderr)


_NC_CACHE = {}


def _get_nc():
    if "nc" not in _NC_CACHE:
        _NC_CACHE["nc"] = build_kernel()
    return _NC_CACHE["nc"]


def kernel(char_ids, emb_table, conv_w, conv_b, trace=False):
    if trace:
        _ensure_ntff_hook()
    nc = _get_nc()
    in_maps = host_prep(char_ids, emb_table, conv_w, conv_b)
    res = run_bass_kernel_spmd(
        nc, in_maps, core_ids=list(range(NCORES)), trace=trace
    )
    # out[f, word] word-linear -> [word, f]
    outs = [res.results[jc]["out"].T for jc in range(NCORES)]
    full = np.concatenate(outs, axis=0).reshape(B, S, F).astype(np.float32)
    if trace:
        return full, res
    return full

